# revision 15
# baseline (speedup 1.0000x reference)
"""Trainium2 Bass kernel for nn_LungCancerGRU (GRU H=64, T=15, B=262144 -> logits [B,2]).

Data parallel over 8 NeuronCores (batch sharded, 32768 rows/core).

Per-core layout: gate units on SBUF partitions, batch on the free dimension.
Batch runs in pair-tiles of 1024 rows = two groups (A, B) of N=512; group A
occupies partitions 0..63, group B 64..127 of every [128, 512] tile.  All
matmul operands are bf16 with CONTIGUOUS rhs streams (fp32 and strided-rhs
matmuls run at a fraction of PE rate - avoid both).

x is transposed on the HOST to [T, BC] so the per-pair x tile loads t-major:
xt [3, 15*512] with xt[g, t*512+n] = x[base+g*512+n, t]; row 2 is a
persistent 1.0 bias lane.  Step t's x-part rhs = xt[:, t*512:(t+1)*512] is
contiguous at base partition 0 (matmul rhs must start at partition 0/32/64).

Per timestep t (per pair-tile), z sign-flipped host-side so one sigmoid
yields both r and w = 1-z:
  psum_rz[:, :512] = BD(W_hr^T) @ h + x3_r @ [x^A; x^B; 1]    (r preact+bias)
  psum_rz[:, 512:] = BD(-W_hz^T) @ h + x3_z @ ...             (-z preact-bias)
  psum_nh          = BD(W_hn^T) @ h                           (h-part of n)
  psum_n           = x3_n @ [x^A; x^B; 1]                     (x w_n + b_ih_n)
  rz  = sigmoid(psum_rz)     ONE ACT op [128,1024] -> [r | w]
  m1  = (psum_nh + b_hhn) * r     fused scalar_tensor_tensor on DVE
  psum_n += I128 @ m1             identity-matmul accumulate (PE)
  n   = tanh(psum_n)              ACT
  u = n - h (GpSimd); v = w*u (DVE); h' = h + v (DVE)

IL=4 pair-tiles run in lockstep so the ~7us per-step dependency chain
pipelines across pairs (PSUM slots recycle mid-chain).  The FC head + output
DMA of superblock b are emitted after step 1 of superblock b+1 so the PE
never stalls on the end-of-superblock chain drain.

Output is stored transposed in DRAM as [2, BC] (contiguous row DMAs) and
transposed back on the host.
"""

import sys

import numpy as np

sys.path.insert(0, "/opt/trn_rl_repo")

B, T, H = 262144, 15, 64
NCORES = 8
BC = B // NCORES          # 32768 rows per core
N = 512                   # batch columns per group
PAIR = 2 * N              # 1024 rows per pair-tile
NPAIR = BC // PAIR        # 32 pair-tiles per core
IL = 4                    # pair-tiles processed in lockstep
NSB = NPAIR // IL         # superblocks
XW = T * N                # xt tile free width (7680)
XSLOTS = 6                # persistent xt buffers

_cache = {}


def _build(dt_h_name):
    from contextlib import ExitStack

    import concourse.bacc as bacc
    import concourse.mybir as mybir
    from concourse.tile import TileContext

    f32 = mybir.dt.float32
    dt_h = getattr(mybir.dt, dt_h_name)
    Act = mybir.ActivationFunctionType
    Alu = mybir.AluOpType

    nc = bacc.Bacc(None)

    x_in = nc.dram_tensor("xT", [T, BC], f32, kind="ExternalInput")
    out_d = nc.dram_tensor("out", [2, BC], f32, kind="ExternalOutput")
    cb_in = nc.dram_tensor("cb", [128, 1024], dt_h, kind="ExternalInput")
    ones_in = nc.dram_tensor("ones3", [3, XW], dt_h, kind="ExternalInput")
    cf_in = nc.dram_tensor("cf", [128, 8], f32, kind="ExternalInput")

    with TileContext(nc) as tc, ExitStack() as es:
        # ---- constants: one DMA per dtype ----
        cpool = es.enter_context(tc.tile_pool(name="const", bufs=1))
        cb = cpool.tile([128, 1024], dt_h)
        nc.sync.dma_start(cb[:], cb_in[:])
        cf = cpool.tile([128, 8], f32)
        nc.sync.dma_start(cf[:], cf_in[:])

        bd = [cb[:, 128 * g:128 * (g + 1)] for g in range(3)]  # r, -z, n
        i128 = cb[:, 384:512]
        wfc = cb[:, 512:514]
        x3 = [cb[32 * g:32 * g + 3, 640:768] for g in range(3)]
        bhn = cf[:, 0:1]
        bfc = cf[0:2, 1:2]

        # ---- pools ----
        xt_pool = es.enter_context(tc.tile_pool(name="xt", bufs=XSLOTS))
        hp = es.enter_context(tc.tile_pool(name="h", bufs=2 * IL))
        hfp = es.enter_context(tc.tile_pool(name="hf", bufs=IL))
        rzp = es.enter_context(tc.tile_pool(name="rz", bufs=2 * IL))
        m1p = es.enter_context(tc.tile_pool(name="m1", bufs=2 * IL))
        nsp = es.enter_context(tc.tile_pool(name="ns", bufs=2 * IL))
        upl = es.enter_context(tc.tile_pool(name="u", bufs=2 * IL))
        vpl = es.enter_context(tc.tile_pool(name="v", bufs=2 * IL))
        m2p = es.enter_context(tc.tile_pool(name="m2", bufs=2 * IL))
        stp = es.enter_context(tc.tile_pool(name="st", bufs=2))
        przp = es.enter_context(tc.tile_pool(name="prz", bufs=2, space="PSUM"))
        pnhp = es.enter_context(tc.tile_pool(name="pnh", bufs=1, space="PSUM"))
        pnp = es.enter_context(tc.tile_pool(name="pn", bufs=3, space="PSUM"))

        def mm(out, lhsT, rhs, start, stop, tp=None):
            nc.tensor.matmul(out, lhsT, rhs, start=start, stop=stop,
                             skip_group_check=True, tile_position=tp)

        # ---- persistent xt tiles; row 2 = 1.0 (bias lane), set once ----
        xts = []
        for i in range(XSLOTS):
            xt = xt_pool.tile([67, XW], dt_h, tag="xt", name="xt")
            # x lives at rows {0:2, 32:34, 64:66} (one copy per PE row-group
            # so the three gate x-matmuls run concurrently row-tiled); rows
            # 2/34/66 are all-ones bias lanes, loaded once via HWDGE (off the
            # SWDGE queue the x DMAs use, and DMA writes have no partition-
            # alignment rule).
            nc.sync.dma_start(xt[2:67:32, :], ones_in[:])
            xts.append(xt)

        # ---- engine warm-ups: fold the const-DMA sems into each engine's
        # clock once so steady-state instructions never re-wait on them.
        pwarm = pnp.tile([2, 2], f32, tag="pn")
        mm(pwarm[:], cb[0:2, 0:2], cb[0:2, 0:2], True, True)
        wt = cpool.tile([2, 8], f32)
        nc.vector.tensor_copy(wt[0:1, 0:1], cf[0:1, 0:1])
        nc.vector.tensor_copy(wt[0:1, 1:2], cb[0:1, 0:1])
        nc.scalar.copy(wt[0:1, 2:3], cf[0:1, 0:1])
        nc.scalar.copy(wt[0:1, 3:4], cb[0:1, 0:1])
        nc.gpsimd.tensor_copy(wt[0:1, 4:5], cb[0:1, 0:1])
        nc.gpsimd.tensor_copy(wt[0:1, 5:6], cf[0:1, 0:1])
        nc.vector.tensor_copy(wt[0:1, 6:7], cf[0:1, 1:2])

        def emit_fc(prs):
            """FC head + one contiguous out-DMA for a finished superblock."""
            st = stp.tile([2, IL * PAIR], f32, tag="st", name="st")
            for pr in prs:
                h = pr["h"]
                j = pr["j"]
                for g in range(2):
                    p_l = pnp.tile([2, N], f32, tag="pn", name="pl")
                    mm(p_l[:], wfc[64 * g:64 * (g + 1), :],
                       h[64 * g:64 * (g + 1), :], True, True)
                    stg = st[0:2, j * PAIR + g * N:j * PAIR + (g + 1) * N]
                    nc.vector.tensor_scalar(stg, p_l[:], bfc, None, Alu.add)
            base = prs[0]["blk"] * IL * PAIR
            nc.sync.dma_start(out_d[:, base:base + IL * PAIR], st[:])

        prev_prs = None
        for blk in range(NSB):
            prs = []
            for j in range(IL):
                p = blk * IL + j
                xt = xts[p % XSLOTS]
                # cast DMAs (f32 -> bf16), 2KB-contiguous runs; one copy per
                # PE row-group band
                src = x_in[:, p * PAIR:(p + 1) * PAIR]
                for band in range(3):
                    nc.gpsimd.dma_start(
                        xt[32 * band:32 * band + 2, :].rearrange(
                            "g (t n) -> g t n", t=T),
                        src.rearrange("t (g n) -> g t n", g=2))
                prs.append({"xt": xt, "h": None, "j": j, "blk": blk})
            for t in range(T):
                first = t == 0
                last = t == T - 1
                xc = slice(t * N, (t + 1) * N)
                for pr in prs:
                    pr["prz"] = przp.tile([128, 2 * N], f32, tag="prz", name="prz")
                    pr["pn"] = pnp.tile([128, N], f32, tag="pn", name="pn")
                # ---- PE: x-part trio first (no h dependency; the three
                # gate matmuls run concurrently in row-groups 0/1/2), then
                # the h-part matmuls accumulate on top ----
                for pr in prs:
                    mm(pr["prz"][:, 0:N], x3[0], pr["xt"][0:3, xc],
                       True, first)
                    mm(pr["prz"][:, N:2 * N], x3[1], pr["xt"][32:35, xc],
                       True, first)
                    mm(pr["pn"][:], x3[2], pr["xt"][64:67, xc], True, False)
                if not first:
                    for pr in prs:
                        mm(pr["prz"][:, 0:N], bd[0], pr["h"][:], False, True)
                    for pr in prs:
                        mm(pr["prz"][:, N:2 * N], bd[1], pr["h"][:], False, True)
                # ---- sigmoid -> [r | w] ----
                for pr in prs:
                    rz = rzp.tile([128, 2 * N], dt_h, tag="rz", name="rz")
                    nc.scalar.activation(rz[:], pr["prz"][:], Act.Sigmoid)
                    pr["rz"] = rz
                if not first:
                    for pr in prs:
                        pr["pnh"] = pnhp.tile([128, N], f32, tag="pnh", name="pnh")
                        mm(pr["pnh"][:], bd[2], pr["h"][:], True, True)
                # ---- m1 = (nh + b_hhn) * r ----
                for pr in prs:
                    m1 = m1p.tile([128, N], dt_h, tag="m1", name="m1")
                    if first:
                        nc.vector.tensor_scalar(m1[:], pr["rz"][:, 0:N], bhn,
                                                None, Alu.mult)
                    else:
                        nc.vector.scalar_tensor_tensor(
                            m1[:], pr["pnh"][:], bhn, pr["rz"][:, 0:N],
                            Alu.add, Alu.mult)
                    pr["m1"] = m1
                for pr in prs:
                    m2 = m2p.tile([128, N], f32, tag="m2", name="m2")
                    nc.vector.scalar_tensor_tensor(
                        m2[:], pr["pn"][:], 0.0, pr["m1"][:],
                        Alu.add, Alu.add)
                    pr["m2"] = m2
                for pr in prs:
                    ns = nsp.tile([128, N], dt_h, tag="ns", name="ns")
                    nc.scalar.activation(ns[:], pr["m2"][:], Act.Tanh)
                    pr["ns"] = ns
                # ---- h' = h + w*(n - h) ----
                for pr in prs:
                    if last:
                        hn = hfp.tile([128, N], dt_h, tag="hf", name="hf")
                    else:
                        hn = hp.tile([128, N], dt_h, tag="h", name="h")
                    if first:
                        nc.gpsimd.tensor_tensor(hn[:], pr["ns"][:],
                                                pr["rz"][:, N:2 * N], Alu.mult)
                    else:
                        u = upl.tile([128, N], dt_h, tag="u", name="u")
                        nc.gpsimd.tensor_tensor(u[:], pr["ns"][:], pr["h"][:],
                                                Alu.subtract)
                        v = vpl.tile([128, N], dt_h, tag="v", name="v")
                        nc.gpsimd.tensor_tensor(v[:], pr["rz"][:, N:2 * N],
                                                u[:], Alu.mult)
                        nc.vector.tensor_tensor(hn[:], pr["h"][:], v[:],
                                                Alu.add)
                    pr["h"] = hn
                # FC of the previous superblock rides in here so the PE
                # keeps streaming while that superblock's chain drains.
                if t == 1 and prev_prs is not None:
                    emit_fc(prev_prs)
                    prev_prs = None
            prev_prs = prs
        emit_fc(prev_prs)

    nc.compile()
    return nc


def _host_constants(W_ih, W_hh, b_ih, b_hh, W_fc, b_fc, dt_h_np):
    f32 = np.float32
    cb = np.zeros((128, 1024), f32)
    cf = np.zeros((128, 8), f32)
    w_in = W_ih[:, 0].astype(f32)
    sgn = [1.0, -1.0, 1.0]
    for g in range(3):
        W = W_hh[64 * g:64 * (g + 1)].astype(f32) * sgn[g]   # [64, 64]
        cb[0:64, 128 * g:128 * g + 64] = W.T
        cb[64:128, 128 * g + 64:128 * g + 128] = W.T
        wg = w_in[64 * g:64 * (g + 1)] * sgn[g]
        row = 32 * g
        cb[row, 640:640 + 64] = wg
        cb[row + 1, 640 + 64:640 + 128] = wg
        if g < 2:
            bias = (b_ih[64 * g:64 * (g + 1)] + b_hh[64 * g:64 * (g + 1)]) * sgn[g]
        else:
            bias = b_ih[128:192]
        cb[row + 2, 640:640 + 128] = np.concatenate([bias, bias])
    cb[:, 384:512] = np.eye(128, dtype=f32)
    cb[0:64, 512:514] = W_fc.T
    cb[64:128, 512:514] = W_fc.T
    cf[:, 0] = np.concatenate([b_hh[128:192]] * 2)
    cf[0:2, 1] = b_fc
    ones3 = np.ones((3, XW), np.float32)
    return {"cb": cb.astype(dt_h_np), "cf": cf, "ones3": ones3.astype(dt_h_np)}


def kernel(x, W_ih, W_hh, b_ih, b_hh, W_fc, b_fc, _trace=False, _trace_kwargs=None):
    import ml_dtypes

    from concourse.bass_utils import run_bass_kernel_spmd

    dt_h_name = "bfloat16"
    if dt_h_name not in _cache:
        _cache[dt_h_name] = _build(dt_h_name)
    nc = _cache[dt_h_name]

    consts = _host_constants(W_ih, W_hh, b_ih, b_hh, W_fc, b_fc,
                             ml_dtypes.bfloat16)
    xT = np.ascontiguousarray(np.asarray(x, np.float32).T)   # [T, B]
    in_maps = []
    for c in range(NCORES):
        m = {"xT": np.ascontiguousarray(xT[:, c * BC:(c + 1) * BC])}
        m.update(consts)
        in_maps.append(m)
    kw = {}
    if _trace:
        kw["trace"] = True
        if _trace_kwargs:
            kw.update(_trace_kwargs)
    res = run_bass_kernel_spmd(nc, in_maps, list(range(NCORES)), **kw)
    out = np.concatenate(
        [np.ascontiguousarray(res.results[c]["out"].T) for c in range(NCORES)],
        axis=0)
    if _trace:
        return out, res
    return out


if __name__ == "__main__":
    rng = np.random.default_rng(0)
    s = 1.0 / np.sqrt(H)
    inputs = {
        "x": rng.standard_normal((B, T), dtype=np.float32),
        "W_ih": rng.uniform(-s, s, (3 * H, 1)).astype(np.float32),
        "W_hh": rng.uniform(-s, s, (3 * H, H)).astype(np.float32),
        "b_ih": rng.uniform(-s, s, (3 * H,)).astype(np.float32),
        "b_hh": rng.uniform(-s, s, (3 * H,)).astype(np.float32),
        "W_fc": rng.uniform(-s, s, (2, H)).astype(np.float32),
        "b_fc": rng.uniform(-s, s, (2,)).astype(np.float32),
    }
    out = kernel(**inputs)
    print(out.shape, out.dtype, out[:4])
